# revision 17
# baseline (speedup 1.0000x reference)
"""Trainium2 Bass kernel for batched displacement-operator construction.

Math: for each alpha_b,
    Da[b] = diag(u) @ (V @ diag(exp(-i r lam)) @ V.T) @ diag(v)
with u_i = w^i, v_j = (1/w)^j, w = i*alpha/|alpha|.  Since u_i*v_j = w^(i-j)
(|w| == 1 up to fp eps), the outer phase factor is a Toeplitz matrix Ph whose
tiles are slices of a per-alpha [128, 1920] shifted-window table, precomputed
on the host.  On device per alpha: 2 real 1024^3 matmuls in bf16 (measured
~23% faster than fp32r on HW, ample precision for the 2e-2 gate), then the
complex elementwise phase multiply (DVE mults from PSUM, Pool add/subs).

Symmetry: M = V E V^T is symmetric and Ph[j,i] = conj(Ph[i,j]), so the
mirror tile Da[j,i] = (m1+m2) + i(m4-m3) reuses the SAME four products
m1..m4 = {C,S}x{pr,pi} computed for Da[i,j] = (m1-m2) + i(m3+m4).  The
lower-left quadrant is therefore built by PE-transposing the finished bf16
mirror planes of the upper-right tiles and DMA-ing them straight from PSUM
-- no extra DVE/Pool/ACT work for 1/4 of the output.

Sharding: 16 alphas data-parallel over 8 cores (2 per core); evecs replicated.
"""

import sys

sys.path.insert(0, "/opt/trn_rl_repo")

import numpy as np

N = 1024
B = 16
NCORES = 8
APC = B // NCORES  # alphas per core
P = 128
KC = N // P  # contraction chunks
MC = N // P  # output row chunks
NT = 512  # matmul free-dim tile (fp32 PSUM bank)
NNT = N // NT  # output col chunks
WWIN = 1920  # phase-window free size
C0 = 896  # phase-window offset constant

_cache = {}


def _build_module(reps=1):
    import contextlib

    import concourse.bacc as bacc
    import concourse.mybir as mybir
    import concourse.tile as tile

    f32 = mybir.dt.float32
    bf16 = mybir.dt.bfloat16

    nc = bacc.Bacc(
        "TRN2",
        target_bir_lowering=False,
        debug=False,
        num_devices=NCORES,
    )

    vt_d = nc.dram_tensor("vt", [N, N], bf16, kind="ExternalInput")
    esc_d = nc.dram_tensor("esc", [P, APC * 2 * KC], f32, kind="ExternalInput")
    ph_d = nc.dram_tensor("ph", [P, APC * 2 * WWIN], f32, kind="ExternalInput")
    outr_d = nc.dram_tensor("outr", [APC, N, N], bf16, kind="ExternalOutput")
    outi_d = nc.dram_tensor("outi", [APC, N, N], bf16, kind="ExternalOutput")

    with tile.TileContext(nc) as tc:
        with (
            tc.tile_pool(name="const", bufs=1) as cpool,
            tc.tile_pool(name="wts", bufs=2) as wpool,
            tc.tile_pool(name="work", bufs=3) as work,
            tc.tile_pool(name="psum2", bufs=2, space="PSUM") as pp,
            tc.tile_pool(name="psum0", bufs=1, space="PSUM") as ppa,
            tc.tile_pool(name="psumb", bufs=1, space="PSUM") as ppb,
        ):
            esc = cpool.tile([P, APC * 2 * KC], f32)
            ph = cpool.tile([P, APC * 2 * WWIN], f32)
            from concourse.masks import make_identity

            ident = cpool.tile([P, P], bf16, name="ident")
            make_identity(nc, ident)

            # Per-chunk tiles so Tile tracks dependencies at chunk
            # granularity.
            vt = [
                cpool.tile([P, N], bf16, tag=f"vt{kc}", name=f"vt{kc}")
                for kc in range(KC)
            ]

            # esc first (tiny, gates all weight scaling); ph goes last on
            # SWDGE since the phase tiles are first consumed later.
            nc.gpsimd.dma_start(esc[:], esc_d[:])
            nc.gpsimd.dma_start(ph[:], ph_d[:])
            for kc in range(KC):
                nc.sync.dma_start(vt[kc][:], vt_d[kc * P : (kc + 1) * P, :])

            rep_ctx = (
                tc.For_i(0, reps, 1) if reps > 1 else contextlib.nullcontext()
            )
            with rep_ctx:
                _emit_body(nc, tc, vt, esc, ph, work, pp, ppa, ppb,
                           outr_d, outi_d, mybir, wpool, ident)

    nc.compile()
    return nc


def _emit_body(nc, tc, vt, esc, ph, work, pp, ppa, ppb, outr_d, outi_d,
               mybir, wpool, ident):
    bf16 = mybir.dt.bfloat16
    f32 = mybir.dt.float32
    Alu = mybir.AluOpType
    Act = mybir.ActivationFunctionType
    HM = MC // 2  # mirror boundary

    # Scale VT rows by er = cos(r*lam) and ei = -sin(r*lam) (per-partition
    # scalars) for BOTH alphas up front on ACT, which is otherwise idle.
    # lc/ls rings have depth 2 so the two alphas' weights coexist and the
    # PE never waits on weight scaling at the alpha boundary.
    lcs = {}
    for a in range(APC):
        for kc in range(KC):
            col_er = a * 2 * KC + kc
            col_ei = a * 2 * KC + KC + kc
            lc = wpool.tile([P, N], bf16, tag=f"lc{kc}", name=f"lc{kc}_{a}")
            ls = wpool.tile([P, N], bf16, tag=f"ls{kc}", name=f"ls{kc}_{a}")
            nc.scalar.activation(
                lc[:], vt[kc][:], Act.Copy, scale=esc[:, col_er : col_er + 1]
            )
            nc.scalar.activation(
                ls[:], vt[kc][:], Act.Copy, scale=esc[:, col_ei : col_ei + 1]
            )
            lcs[(a, kc)] = (lc, ls)

    for a in range(APC):
        base_c = (a * 2) * WWIN
        base_s = (a * 2 + 1) * WWIN

        dmr = {}
        dmi = {}

        def mirror_flush(pdr, pdi, targets):
            """Copy packed transposed mirror tiles PSUM->SBUF (ACT) and DMA
            each [P,128] slot to its (row-block, col-block) home."""
            wtot = 128 * len(targets)
            sdr = work.tile([P, NT], bf16, tag="sdr")
            sdi = work.tile([P, NT], bf16, tag="sdi")
            nc.scalar.activation(sdr[:, 0:wtot], pdr[:, 0:wtot], Act.Copy)
            nc.scalar.activation(sdi[:, 0:wtot], pdi[:, 0:wtot], Act.Copy)
            for s, (rb, cb) in enumerate(targets):
                nc.sync.dma_start(
                    outr_d[a, rb * P : (rb + 1) * P, cb * P : (cb + 1) * P],
                    sdr[:, s * P : (s + 1) * P],
                )
                nc.sync.dma_start(
                    outi_d[a, rb * P : (rb + 1) * P, cb * P : (cb + 1) * P],
                    sdi[:, s * P : (s + 1) * P],
                )

        for m in range(MC):
            i = m if m < HM else m - HM  # block row within its quadrant
            wq = (i + 1) * P  # computed (block-lower-triangle) width
            if m < HM:
                pc0 = ppa.tile([P, NT], f32, tag="pc0")
                ps0 = ppa.tile([P, NT], f32, tag="ps0")
            pc1 = pp.tile([P, NT], f32, tag="pc1")
            ps1 = pp.tile([P, NT], f32, tag="ps1")

            if m >= HM:
                # Lower-left quadrant: transpose the finished bf16 mirror
                # planes of tiles (m', n=1), full width.
                q = m - HM
                pdr = ppb.tile([P, NT], bf16, tag="pdr")
                pdi = ppb.tile([P, NT], bf16, tag="pdi")
                for mp in range(HM):
                    nc.tensor.matmul(
                        pdr[:, mp * P : (mp + 1) * P],
                        dmr[mp][:, q * P : (q + 1) * P], ident[:],
                        is_transpose=True, start=True, stop=True,
                    )
                    nc.tensor.matmul(
                        pdi[:, mp * P : (mp + 1) * P],
                        dmi[mp][:, q * P : (q + 1) * P], ident[:],
                        is_transpose=True, start=True, stop=True,
                    )
                # DMA cannot read PSUM: bounce through SBUF on ACT.
                sdr = work.tile([P, NT], bf16, tag="sdr")
                sdi = work.tile([P, NT], bf16, tag="sdi")
                nc.scalar.activation(sdr[:], pdr[:], Act.Copy)
                nc.scalar.activation(sdi[:], pdi[:], Act.Copy)
                nc.sync.dma_start(
                    outr_d[a, m * P : (m + 1) * P, 0:NT], sdr[:]
                )
                nc.sync.dma_start(
                    outi_d[a, m * P : (m + 1) * P, 0:NT], sdi[:]
                )

            # Both diagonal quadrants: compute only the block-lower-
            # triangle (narrowed streams); the upper tiles are derived
            # from this iteration's products below.  The UNSCALED vt
            # block is the stationary operand so one weight load serves
            # all streams of a chunk.
            nn = 0 if m < HM else 1
            cbase = nn * NT  # quadrant column base
            for kc in range(KC):
                lc, ls = lcs[(a, kc)]
                wap = vt[kc][:, m * P : (m + 1) * P]
                st = kc == 0
                sp = kc == KC - 1
                if m < HM:
                    # diagonal quadrant stream (narrowed) + full n=1
                    nc.tensor.matmul(pc0[:, 0:wq], wap, lc[:, 0:wq],
                                     start=st, stop=sp)
                    nc.tensor.matmul(ps0[:, 0:wq], wap, ls[:, 0:wq],
                                     start=st, stop=sp)
                    nc.tensor.matmul(pc1[:], wap, lc[:, NT:N],
                                     start=st, stop=sp)
                    nc.tensor.matmul(ps1[:], wap, ls[:, NT:N],
                                     start=st, stop=sp)
                else:
                    nc.tensor.matmul(pc1[:, 0:wq], wap,
                                     lc[:, NT : NT + wq],
                                     start=st, stop=sp)
                    nc.tensor.matmul(ps1[:, 0:wq], wap,
                                     ls[:, NT : NT + wq],
                                     start=st, stop=sp)

            for n in range(NNT):
                if m >= HM and n == 0:
                    continue
                diag = n == nn  # narrowed diagonal-quadrant tile
                w = wq if diag else NT
                pc = pc0 if (m < HM and n == 0) else pc1
                ps = ps0 if (m < HM and n == 0) else ps1
                t0 = C0 - P * m + NT * n
                pr = ph[:, base_c + t0 : base_c + t0 + w]
                pi = ph[:, base_s + t0 : base_s + t0 + w]
                m1 = work.tile([P, NT], f32, tag="m1")
                m2 = work.tile([P, NT], f32, tag="m2")
                m3 = work.tile([P, NT], f32, tag="m3")
                m4 = work.tile([P, NT], f32, tag="m4")
                nc.vector.tensor_tensor(m1[:, 0:w], pc[:, 0:w], pr, Alu.mult)
                nc.vector.tensor_tensor(m2[:, 0:w], ps[:, 0:w], pi, Alu.mult)
                nc.vector.tensor_tensor(m3[:, 0:w], pc[:, 0:w], pi, Alu.mult)
                nc.vector.tensor_tensor(m4[:, 0:w], ps[:, 0:w], pr, Alu.mult)
                dar = work.tile([P, NT], bf16, tag="dar")
                dai = work.tile([P, NT], bf16, tag="dai")
                nc.gpsimd.tensor_tensor(dar[:, 0:w], m1[:, 0:w], m2[:, 0:w],
                                        Alu.subtract)
                nc.gpsimd.tensor_tensor(dai[:, 0:w], m3[:, 0:w], m4[:, 0:w],
                                        Alu.add)
                if m < HM and n == 1:
                    # Mirror planes for the (m+HM, n=0) tiles: the same
                    # four products, opposite sign combinations.
                    darm = wpool.tile([P, NT], bf16, tag=f"darm{m}",
                                      name=f"darm{m}_{a}")
                    daim = wpool.tile([P, NT], bf16, tag=f"daim{m}",
                                      name=f"daim{m}_{a}")
                    nc.vector.tensor_tensor(darm[:], m1[:], m2[:], Alu.add)
                    nc.vector.tensor_tensor(daim[:], m4[:], m3[:],
                                            Alu.subtract)
                    dmr[m] = darm
                    dmi[m] = daim
                nc.sync.dma_start(
                    outr_d[a, m * P : (m + 1) * P,
                           n * NT : n * NT + w],
                    dar[:, 0:w],
                )
                nc.sync.dma_start(
                    outi_d[a, m * P : (m + 1) * P,
                           n * NT : n * NT + w],
                    dai[:, 0:w],
                )
                if diag and i > 0:
                    # In-quadrant mirrors: derive tiles (j, i) of this
                    # quadrant from slices of this tile's products.
                    tdr = work.tile([P, NT], bf16, tag="tdr")
                    tdi = work.tile([P, NT], bf16, tag="tdi")
                    for j in range(i):
                        sl = slice(j * P, (j + 1) * P)
                        nc.vector.tensor_tensor(tdr[:, sl], m1[:, sl],
                                                m2[:, sl], Alu.add)
                        nc.vector.tensor_tensor(tdi[:, sl], m4[:, sl],
                                                m3[:, sl], Alu.subtract)
                    pqr = ppb.tile([P, NT], bf16, tag="pdr")
                    pqi = ppb.tile([P, NT], bf16, tag="pdi")
                    for j in range(i):
                        nc.tensor.matmul(
                            pqr[:, j * P : (j + 1) * P],
                            tdr[:, j * P : (j + 1) * P], ident[:],
                            is_transpose=True, start=True, stop=True,
                        )
                        nc.tensor.matmul(
                            pqi[:, j * P : (j + 1) * P],
                            tdi[:, j * P : (j + 1) * P], ident[:],
                            is_transpose=True, start=True, stop=True,
                        )
                    qrow = 0 if m < HM else HM
                    mirror_flush(
                        pqr, pqi,
                        [(qrow + j, nn * HM + i) for j in range(i)],
                    )


def _get_module():
    if "nc" not in _cache:
        _cache["nc"] = _build_module()
    return _cache["nc"]


def _host_precompute(alpha_real, alpha_imag, evals):
    """Per-alpha scalar tables, mirroring the reference's fp32 arithmetic."""
    ar = np.asarray(alpha_real, np.float32)
    ai = np.asarray(alpha_imag, np.float32)
    ev = np.asarray(evals, np.float32)

    esc_all = np.empty((B, 2, KC, P), np.float32)  # (b, er/ei, kc, p)
    ph_all = np.empty((B, 2, P, WWIN), np.float32)  # (b, re/im, p, w)

    prow = np.arange(P)[:, None]
    scol = np.arange(WWIN)[None, :]
    idx = (prow - scol) + C0 + (N - 1)  # into d-table of length 2N-1

    for b in range(B):
        alpha = np.complex64(complex(ar[b], ai[b]))
        r = np.float32(np.abs(alpha)) + np.float32(1e-10)
        eit = np.complex64(alpha / r)
        w = np.complex128(1j) * np.complex128(eit)

        t32 = (np.float32(r) * ev).astype(np.float32)
        t64 = t32.astype(np.float64)
        er = np.cos(t64).astype(np.float32)
        ei = (-np.sin(t64)).astype(np.float32)
        esc_all[b, 0] = er.reshape(KC, P)
        esc_all[b, 1] = ei.reshape(KC, P)

        d = np.arange(-(N - 1), N)
        ptab = w ** d  # complex128, |w|~1 so no overflow
        wc = ptab.real.astype(np.float32)
        ws = ptab.imag.astype(np.float32)
        ph_all[b, 0] = wc[idx]
        ph_all[b, 1] = ws[idx]

    return esc_all, ph_all


def _make_in_maps(alpha_real, alpha_imag, evals, evecs):
    import ml_dtypes

    bf = ml_dtypes.bfloat16
    evecs_f = np.ascontiguousarray(np.asarray(evecs, np.float32))
    vt_np = np.ascontiguousarray(evecs_f.T.astype(bf))
    esc_all, ph_all = _host_precompute(alpha_real, alpha_imag, evals)

    in_maps = []
    for c in range(NCORES):
        bs = [c * APC + a for a in range(APC)]
        # esc columns: per alpha [er cols | ei cols]; value at (p, col) with
        # col = a*2*KC + which*KC + kc  ->  esc_all[b, which, kc, p]
        esc = np.empty((P, APC * 2 * KC), np.float32)
        ph = np.empty((P, APC * 2 * WWIN), np.float32)
        for a, b in enumerate(bs):
            for which in range(2):
                cols = a * 2 * KC + which * KC
                esc[:, cols : cols + KC] = esc_all[b, which].T
                wbase = (a * 2 + which) * WWIN
                ph[:, wbase : wbase + WWIN] = ph_all[b, which]
        in_maps.append({"vt": vt_np, "esc": esc, "ph": ph})
    return in_maps


def kernel(alpha_real, alpha_imag, evals, evecs):
    from concourse import bass_utils

    nc = _get_module()
    in_maps = _make_in_maps(alpha_real, alpha_imag, evals, evecs)

    res = bass_utils.run_bass_kernel_spmd(
        nc, in_maps, core_ids=list(range(NCORES))
    )

    out = np.empty((B, N, N), np.complex64)
    for c in range(NCORES):
        outr = res.results[c]["outr"]
        outi = res.results[c]["outi"]
        for a in range(APC):
            b = c * APC + a
            out.real[b] = np.asarray(outr[a], np.float32)
            out.imag[b] = np.asarray(outi[a], np.float32)
    return out


# revision 18
# speedup vs baseline: 4.2217x; 4.2217x over previous
"""Trainium2 Bass kernel for batched displacement-operator construction.

Math: for each alpha_b,
    Da[b] = diag(u) @ (V @ diag(exp(-i r lam)) @ V.T) @ diag(v)
with u_i = w^i, v_j = (1/w)^j, w = i*alpha/|alpha|.  Since u_i*v_j = w^(i-j)
(|w| == 1 up to fp eps), the outer phase factor is a Toeplitz matrix Ph whose
tiles are slices of a per-alpha [128, 1920] shifted-window table, precomputed
on the host.  On device per alpha: 2 real 1024^3 matmuls in bf16 (measured
~23% faster than fp32r on HW, ample precision for the 2e-2 gate), then the
complex elementwise phase multiply (DVE mults from PSUM, Pool add/subs).

Symmetry: M = V E V^T is symmetric and Ph[j,i] = conj(Ph[i,j]), so the
mirror tile Da[j,i] = (m1+m2) + i(m4-m3) reuses the SAME four products
m1..m4 = {C,S}x{pr,pi} computed for Da[i,j] = (m1-m2) + i(m3+m4).  The
lower-left quadrant is therefore built by PE-transposing the finished bf16
mirror planes of the upper-right tiles and DMA-ing them straight from PSUM
-- no extra DVE/Pool/ACT work for 1/4 of the output.

Sharding: 16 alphas data-parallel over 8 cores (2 per core); evecs replicated.
"""

import sys

sys.path.insert(0, "/opt/trn_rl_repo")

import numpy as np

N = 1024
B = 16
NCORES = 8
APC = B // NCORES  # alphas per core
P = 128
KC = N // P  # contraction chunks
MC = N // P  # output row chunks
NT = 512  # matmul free-dim tile (fp32 PSUM bank)
NNT = N // NT  # output col chunks
WWIN = 1920  # phase-window free size
C0 = 896  # phase-window offset constant

_cache = {}


def _build_module(reps=1):
    import contextlib

    import concourse.bacc as bacc
    import concourse.mybir as mybir
    import concourse.tile as tile

    f32 = mybir.dt.float32
    bf16 = mybir.dt.bfloat16

    nc = bacc.Bacc(
        "TRN2",
        target_bir_lowering=False,
        debug=False,
        num_devices=NCORES,
    )

    vt_d = nc.dram_tensor("vt", [N, N], bf16, kind="ExternalInput")
    esc_d = nc.dram_tensor("esc", [P, APC * 2 * KC], f32, kind="ExternalInput")
    ph_d = nc.dram_tensor("ph", [P, APC * 2 * WWIN], f32, kind="ExternalInput")
    outr_d = nc.dram_tensor("outr", [APC, N, N], bf16, kind="ExternalOutput")
    outi_d = nc.dram_tensor("outi", [APC, N, N], bf16, kind="ExternalOutput")

    with tile.TileContext(nc) as tc:
        with (
            tc.tile_pool(name="const", bufs=1) as cpool,
            tc.tile_pool(name="wts", bufs=2) as wpool,
            tc.tile_pool(name="work", bufs=3) as work,
            tc.tile_pool(name="psum2", bufs=2, space="PSUM") as pp,
            tc.tile_pool(name="psum0", bufs=1, space="PSUM") as ppa,
            tc.tile_pool(name="psumb", bufs=1, space="PSUM") as ppb,
        ):
            esc = cpool.tile([P, APC * 2 * KC], f32)
            ph = cpool.tile([P, APC * 2 * WWIN], f32)
            from concourse.masks import make_identity

            ident = cpool.tile([P, P], bf16, name="ident")
            make_identity(nc, ident)

            # Per-chunk tiles so Tile tracks dependencies at chunk
            # granularity.
            vt = [
                cpool.tile([P, N], bf16, tag=f"vt{kc}", name=f"vt{kc}")
                for kc in range(KC)
            ]

            # esc first (tiny, gates all weight scaling); ph goes last on
            # SWDGE since the phase tiles are first consumed later.
            nc.gpsimd.dma_start(esc[:], esc_d[:])
            nc.gpsimd.dma_start(ph[:], ph_d[:])
            for kc in range(KC):
                nc.sync.dma_start(vt[kc][:], vt_d[kc * P : (kc + 1) * P, :])

            rep_ctx = (
                tc.For_i(0, reps, 1) if reps > 1 else contextlib.nullcontext()
            )
            with rep_ctx:
                _emit_body(nc, tc, vt, esc, ph, work, pp, ppa, ppb,
                           outr_d, outi_d, mybir, wpool, ident)

    nc.compile()
    return nc


def _emit_body(nc, tc, vt, esc, ph, work, pp, ppa, ppb, outr_d, outi_d,
               mybir, wpool, ident):
    bf16 = mybir.dt.bfloat16
    f32 = mybir.dt.float32
    Alu = mybir.AluOpType
    Act = mybir.ActivationFunctionType
    HM = MC // 2  # mirror boundary

    # Scale VT rows by er = cos(r*lam) and ei = -sin(r*lam) (per-partition
    # scalars) for BOTH alphas up front on ACT, which is otherwise idle.
    # lc/ls rings have depth 2 so the two alphas' weights coexist and the
    # PE never waits on weight scaling at the alpha boundary.
    lcs = {}
    for a in range(APC):
        for kc in range(KC):
            col_er = a * 2 * KC + kc
            col_ei = a * 2 * KC + KC + kc
            lc = wpool.tile([P, N], bf16, tag=f"lc{kc}", name=f"lc{kc}_{a}")
            ls = wpool.tile([P, N], bf16, tag=f"ls{kc}", name=f"ls{kc}_{a}")
            nc.scalar.activation(
                lc[:], vt[kc][:], Act.Copy, scale=esc[:, col_er : col_er + 1]
            )
            nc.scalar.activation(
                ls[:], vt[kc][:], Act.Copy, scale=esc[:, col_ei : col_ei + 1]
            )
            lcs[(a, kc)] = (lc, ls)

    for a in range(APC):
        base_c = (a * 2) * WWIN
        base_s = (a * 2 + 1) * WWIN

        dmr = {}
        dmi = {}
        for m in range(MC):
            if m < HM:
                pc0 = ppa.tile([P, NT], f32, tag="pc0")
                ps0 = ppa.tile([P, NT], f32, tag="ps0")
            pc1 = pp.tile([P, NT], f32, tag="pc1")
            ps1 = pp.tile([P, NT], f32, tag="ps1")

            if m >= HM:
                # Mirror quadrant: transpose the finished bf16 mirror
                # planes of tiles (m', n=1) and DMA straight from PSUM.
                q = m - HM
                pdr = ppb.tile([P, NT], bf16, tag="pdr")
                pdi = ppb.tile([P, NT], bf16, tag="pdi")
                for mp in range(HM):
                    nc.tensor.matmul(
                        pdr[:, mp * P : (mp + 1) * P],
                        dmr[mp][:, q * P : (q + 1) * P], ident[:],
                        is_transpose=True, start=True, stop=True,
                    )
                    nc.tensor.matmul(
                        pdi[:, mp * P : (mp + 1) * P],
                        dmi[mp][:, q * P : (q + 1) * P], ident[:],
                        is_transpose=True, start=True, stop=True,
                    )
                # DMA cannot read PSUM: bounce through SBUF on ACT.
                sdr = work.tile([P, NT], bf16, tag="sdr")
                sdi = work.tile([P, NT], bf16, tag="sdi")
                nc.scalar.activation(sdr[:], pdr[:], Act.Copy)
                nc.scalar.activation(sdi[:], pdi[:], Act.Copy)
                nc.sync.dma_start(
                    outr_d[a, m * P : (m + 1) * P, 0:NT], sdr[:]
                )
                nc.sync.dma_start(
                    outi_d[a, m * P : (m + 1) * P, 0:NT], sdi[:]
                )

            # The UNSCALED vt block is the stationary operand so one
            # weight load serves all streams of a chunk.
            for kc in range(KC):
                lc, ls = lcs[(a, kc)]
                wap = vt[kc][:, m * P : (m + 1) * P]
                st = kc == 0
                sp = kc == KC - 1
                if m < HM:
                    nc.tensor.matmul(pc0[:], wap, lc[:, 0:NT],
                                     start=st, stop=sp)
                    nc.tensor.matmul(ps0[:], wap, ls[:, 0:NT],
                                     start=st, stop=sp)
                nc.tensor.matmul(pc1[:], wap, lc[:, NT:N],
                                 start=st, stop=sp)
                nc.tensor.matmul(ps1[:], wap, ls[:, NT:N],
                                 start=st, stop=sp)

            for n in range(NNT):
                if m >= HM and n == 0:
                    continue
                pc = pc0 if n == 0 else pc1
                ps = ps0 if n == 0 else ps1
                t0 = C0 - P * m + NT * n
                pr = ph[:, base_c + t0 : base_c + t0 + NT]
                pi = ph[:, base_s + t0 : base_s + t0 + NT]
                m1 = work.tile([P, NT], f32, tag="m1")
                m2 = work.tile([P, NT], f32, tag="m2")
                m3 = work.tile([P, NT], f32, tag="m3")
                m4 = work.tile([P, NT], f32, tag="m4")
                nc.vector.tensor_tensor(m1[:], pc[:], pr, Alu.mult)
                nc.vector.tensor_tensor(m2[:], ps[:], pi, Alu.mult)
                nc.vector.tensor_tensor(m3[:], pc[:], pi, Alu.mult)
                nc.vector.tensor_tensor(m4[:], ps[:], pr, Alu.mult)
                dar = work.tile([P, NT], bf16, tag="dar")
                dai = work.tile([P, NT], bf16, tag="dai")
                nc.gpsimd.tensor_tensor(dar[:], m1[:], m2[:], Alu.subtract)
                nc.gpsimd.tensor_tensor(dai[:], m3[:], m4[:], Alu.add)
                if m < HM and n == 1:
                    # Mirror planes for the (m+HM, n=0) tiles: the same
                    # four products, opposite sign combinations.
                    darm = wpool.tile([P, NT], bf16, tag=f"darm{m}",
                                      name=f"darm{m}_{a}")
                    daim = wpool.tile([P, NT], bf16, tag=f"daim{m}",
                                      name=f"daim{m}_{a}")
                    nc.vector.tensor_tensor(darm[:], m1[:], m2[:], Alu.add)
                    nc.vector.tensor_tensor(daim[:], m4[:], m3[:],
                                            Alu.subtract)
                    dmr[m] = darm
                    dmi[m] = daim
                nc.sync.dma_start(
                    outr_d[a, m * P : (m + 1) * P, n * NT : (n + 1) * NT],
                    dar[:],
                )
                nc.sync.dma_start(
                    outi_d[a, m * P : (m + 1) * P, n * NT : (n + 1) * NT],
                    dai[:],
                )


def _get_module():
    if "nc" not in _cache:
        _cache["nc"] = _build_module()
    return _cache["nc"]


def _host_precompute(alpha_real, alpha_imag, evals):
    """Per-alpha scalar tables, mirroring the reference's fp32 arithmetic."""
    ar = np.asarray(alpha_real, np.float32)
    ai = np.asarray(alpha_imag, np.float32)
    ev = np.asarray(evals, np.float32)

    esc_all = np.empty((B, 2, KC, P), np.float32)  # (b, er/ei, kc, p)
    ph_all = np.empty((B, 2, P, WWIN), np.float32)  # (b, re/im, p, w)

    prow = np.arange(P)[:, None]
    scol = np.arange(WWIN)[None, :]
    idx = (prow - scol) + C0 + (N - 1)  # into d-table of length 2N-1

    for b in range(B):
        alpha = np.complex64(complex(ar[b], ai[b]))
        r = np.float32(np.abs(alpha)) + np.float32(1e-10)
        eit = np.complex64(alpha / r)
        w = np.complex128(1j) * np.complex128(eit)

        t32 = (np.float32(r) * ev).astype(np.float32)
        t64 = t32.astype(np.float64)
        er = np.cos(t64).astype(np.float32)
        ei = (-np.sin(t64)).astype(np.float32)
        esc_all[b, 0] = er.reshape(KC, P)
        esc_all[b, 1] = ei.reshape(KC, P)

        d = np.arange(-(N - 1), N)
        ptab = w ** d  # complex128, |w|~1 so no overflow
        wc = ptab.real.astype(np.float32)
        ws = ptab.imag.astype(np.float32)
        ph_all[b, 0] = wc[idx]
        ph_all[b, 1] = ws[idx]

    return esc_all, ph_all


def _make_in_maps(alpha_real, alpha_imag, evals, evecs):
    import ml_dtypes

    bf = ml_dtypes.bfloat16
    evecs_f = np.ascontiguousarray(np.asarray(evecs, np.float32))
    vt_np = np.ascontiguousarray(evecs_f.T.astype(bf))
    esc_all, ph_all = _host_precompute(alpha_real, alpha_imag, evals)

    in_maps = []
    for c in range(NCORES):
        bs = [c * APC + a for a in range(APC)]
        # esc columns: per alpha [er cols | ei cols]; value at (p, col) with
        # col = a*2*KC + which*KC + kc  ->  esc_all[b, which, kc, p]
        esc = np.empty((P, APC * 2 * KC), np.float32)
        ph = np.empty((P, APC * 2 * WWIN), np.float32)
        for a, b in enumerate(bs):
            for which in range(2):
                cols = a * 2 * KC + which * KC
                esc[:, cols : cols + KC] = esc_all[b, which].T
                wbase = (a * 2 + which) * WWIN
                ph[:, wbase : wbase + WWIN] = ph_all[b, which]
        in_maps.append({"vt": vt_np, "esc": esc, "ph": ph})
    return in_maps


def kernel(alpha_real, alpha_imag, evals, evecs):
    from concourse import bass_utils

    nc = _get_module()
    in_maps = _make_in_maps(alpha_real, alpha_imag, evals, evecs)

    res = bass_utils.run_bass_kernel_spmd(
        nc, in_maps, core_ids=list(range(NCORES))
    )

    out = np.empty((B, N, N), np.complex64)
    for c in range(NCORES):
        outr = res.results[c]["outr"]
        outi = res.results[c]["outi"]
        for a in range(APC):
            b = c * APC + a
            out.real[b] = np.asarray(outr[a], np.float32)
            out.imag[b] = np.asarray(outi[a], np.float32)
    return out
